# revision 5
# baseline (speedup 1.0000x reference)
"""Trainium2 Bass kernel for nn_ALS_net (embedding_lookup):
out[i] = dot(user_matrix[location[i,0], :], goods_matrix[:, location[i,1]])

Strategy (8 NeuronCores, SPMD):
  - Data-parallel over the 1M location pairs: 125K pairs per core.
  - Both factor tables replicated per core in HBM as [100000, 64] f32 row-major
    (goods transposed on host - layout marshalling).
  - dma_gather (custom SWDGE gather instruction) needs int16 row indices, so
    each table is split into 4 chunks of 25000 rows; pairs are bucketed by the
    (user-chunk, goods-chunk) combination -> 16 joint buckets per core. Within
    a bucket both gathers use dense int16 local indices; buckets are padded to
    a common capacity with trailing -1 (trimmed for free by the gather ucode).
    Each gather instruction is limited to 1024 indices (64 descriptors per
    SDMA lane fits one packet), so buckets issue ceil(cap/1024) sub-gathers.
  - Per bucket: gather user rows + goods rows (256B each) into SBUF,
    DVE elementwise multiply + free-dim reduce -> one f32 dot per pair.
  - Perf: SWDGE descriptor generation (~8ns/index) is the bottleneck, and
    each dma_gather's ucode runs on only the pair of gpsimd DSP cores
    selected by queue_num. Sub-gathers therefore round-robin over all 4
    SWDGE queues (num_swdge_queues=4) so descriptor generation for 4
    gathers runs concurrently on 4 distinct core pairs (~4x the serial
    rate: 2.34ms -> 0.59ms). 4096-index multi-packet sub-gathers amortize
    per-instruction dispatch; buckets triple-buffer in SBUF; the gidx DMA
    overlaps the first bucket's descriptor generation.
  - Host inverts the bucket permutation when gluing shard outputs.
"""
import numpy as np

import concourse.bass as bass
import concourse.bacc as bacc
import concourse.mybir as mybir
from concourse.bass_utils import run_bass_kernel_spmd
from concourse.library_config import mlp

N_PAIRS = 1_000_000
K = 64
NROWS = 100_000          # index range per problem spec (both tables)
N_CHUNK = 4              # table chunks (int16 index reach)
CHUNK = NROWS // N_CHUNK  # 25000 rows per chunk
NB = N_CHUNK * N_CHUNK   # 16 joint buckets
N_CORES = 8
P = 128
NP_CORE = N_PAIRS // N_CORES          # 125000
SUBCAP = 4096            # indices per dma_gather (multi-packet, 4 queues)
TAILCAP = 1024           # last bucket: smaller gathers shrink final DMA drain

TRACE = False            # test.py sets True to capture a profile
LAST_RESULT = None       # BassKernelResults of the last run (for test.py)

_graph_cache = {}


def _build_graph(cap):
    """cap: per-bucket padded index capacity (multiple of SUBCAP)."""
    cap128 = cap // P
    cols16 = cap // 16
    nsub = cap // SUBCAP
    sub128 = SUBCAP // P      # 8 dst rows per sub-gather
    sub16 = SUBCAP // 16      # 64 idx cols per sub-gather
    nc = bacc.Bacc("TRN2", target_bir_lowering=False, debug=False,
                   enable_asserts=False, num_devices=N_CORES,
                   num_swdge_queues=4,
                   dynamic_dma_scratch_size=32768)
    user_d = nc.dram_tensor("user_tab", [NROWS, K], mybir.dt.float32, kind="ExternalInput").ap()
    goods_d = nc.dram_tensor("goods_tab", [NROWS, K], mybir.dt.float32, kind="ExternalInput").ap()
    uidx_d = nc.dram_tensor("uidx", [P, NB * cols16], mybir.dt.int16, kind="ExternalInput").ap()
    gidx_d = nc.dram_tensor("gidx", [P, NB * cols16], mybir.dt.int16, kind="ExternalInput").ap()
    out_d = nc.dram_tensor("out", [P, NB * cap128], mybir.dt.float32, kind="ExternalOutput").ap()

    with nc.Block() as block, \
         nc.sbuf_tensor("uidx_sb", [P, NB * cols16], mybir.dt.int16) as uidx_sb, \
         nc.sbuf_tensor("gidx_sb", [P, NB * cols16], mybir.dt.int16) as gidx_sb, \
         nc.sbuf_tensor("u_t", [P, 3, cap128, K], mybir.dt.float32) as u_t, \
         nc.sbuf_tensor("g_t", [P, 3, cap128, K], mybir.dt.float32) as g_t, \
         nc.sbuf_tensor("prod", [P, cap128, K], mybir.dt.float32) as prod, \
         nc.sbuf_tensor("out_sb", [P, NB * cap128], mybir.dt.float32) as out_sb, \
         nc.semaphore("io") as io, \
         nc.semaphore("gat0") as gat0, \
         nc.semaphore("gat1") as gat1, \
         nc.semaphore("gat2") as gat2, \
         nc.semaphore("cmp") as cmp, \
         nc.semaphore("red") as red:

        gat = (gat0, gat1, gat2)

        @block.sync
        def _(sync: bass.BassEngine):
            sync.dma_start(out=uidx_sb[:], in_=uidx_d[:]).then_inc(io, 16)
            sync.dma_start(out=gidx_sb[:], in_=gidx_d[:]).then_inc(io, 16)

        @block.gpsimd
        def _(gpsimd: bass.BassGpSimd):
            gpsimd.load_library(mlp)
            gpsimd.wait_ge(io, 16)   # uidx loaded; gidx DMA overlaps 1st desc-gen
            for b in range(NB):
                cu, cg = divmod(b, N_CHUNK)
                if b == 0:
                    pass
                elif b == 1:
                    gpsimd.wait_ge(io, 32)
                elif b >= 3:
                    gpsimd.wait_ge(cmp, b - 2)   # mul of bucket b-3 done
                for ti, (tab_ap, c, idx_sb, dst) in enumerate((
                        (user_d, cu, uidx_sb, u_t),
                        (goods_d, cg, gidx_sb, g_t))):
                    tab_c = tab_ap[c * CHUNK:(c + 1) * CHUNK, :]
                    if b == 0 and ti == 1:
                        gpsimd.wait_ge(io, 32)
                    sc = TAILCAP if b == NB - 1 else SUBCAP
                    s128, s16 = sc // P, sc // 16
                    for j in range(cap // sc):
                        qn = (ti * (cap // sc) + j) % 4
                        gpsimd.dma_gather(
                            dst[:, b % 3, j * s128:(j + 1) * s128],
                            tab_c,
                            idx_sb[:, b * cols16 + j * s16:
                                      b * cols16 + (j + 1) * s16],
                            sc, sc, K,
                            single_packet=False,
                            queue_num=qn,
                        ).then_inc(gat[b % 3], 16)
            gpsimd.wait_ge(red, NB)
            gpsimd.dma_start(out=out_d[:], in_=out_sb[:]).then_inc(io, 16)
            gpsimd.wait_ge(io, 48)

        @block.vector
        def _(vector: bass.BassVectorEngine):
            gat_tot = [0, 0, 0]
            for b in range(NB):
                sc = TAILCAP if b == NB - 1 else SUBCAP
                gat_tot[b % 3] += 2 * (cap // sc) * 16
                vector.wait_ge(gat[b % 3], gat_tot[b % 3])
                vector.tensor_tensor(out=prod[:], in0=u_t[:, b % 3], in1=g_t[:, b % 3],
                                     op=mybir.AluOpType.mult).then_inc(cmp, 1)
                vector.tensor_reduce(out=out_sb[:, b * cap128:(b + 1) * cap128],
                                     in_=prod[:], axis=mybir.AxisListType.X,
                                     op=mybir.AluOpType.add).then_inc(red, 1)
    nc.compile()
    return nc


def _shard_core(uidx, gidx, cap):
    """Bucket + sort one core's pairs. Returns (uidx16, gidx16, order, counts)."""
    cols16 = cap // 16
    a = uidx.astype(np.int64)
    g = gidx.astype(np.int64)
    bucket = (a // CHUNK) * N_CHUNK + (g // CHUNK)
    order = np.lexsort((a, bucket))          # by bucket, then user idx (HBM locality)
    counts = np.bincount(bucket, minlength=NB)
    assert counts.max() <= cap, (counts.max(), cap)
    au = (a[order] % CHUNK).astype(np.int16)
    ag = (g[order] % CHUNK).astype(np.int16)
    u16 = np.empty((P, NB * cols16), np.int16)
    g16 = np.empty((P, NB * cols16), np.int16)
    start = 0
    for b in range(NB):
        n = counts[b]
        for src, dst in ((au, u16), (ag, g16)):
            # pad with row 0 (NOT -1: an all-(-1) sub-gather crashes the ucode)
            flat = np.zeros(cap, np.int16)
            flat[:n] = src[start:start + n]
            # per sub-gather wrapped layout: position i of sub j lives at
            # [partition i%16 (replicated x8), col j*(sc//16) + i//16]
            sc = TAILCAP if b == NB - 1 else SUBCAP
            w = flat.reshape(cap // sc, sc // 16, 16)
            w = np.concatenate([w[j].T for j in range(cap // sc)], axis=1)
            dst[:, b * cols16:(b + 1) * cols16] = np.tile(w, (8, 1))
        start += n
    return u16, g16, order, counts


def kernel(location, user_matrix, goods_matrix):
    global LAST_RESULT
    loc = np.asarray(location)
    uidx = np.ascontiguousarray(loc[:, 0]).astype(np.int64, copy=False)
    gidx = np.ascontiguousarray(loc[:, 1]).astype(np.int64, copy=False)
    user_rows = np.ascontiguousarray(np.asarray(user_matrix)[:NROWS], dtype=np.float32)
    goods_rows = np.ascontiguousarray(np.asarray(goods_matrix).T, dtype=np.float32)

    # per-bucket capacity: max count over all cores/buckets, rounded to SUBCAP
    all_bucket = (uidx // CHUNK) * N_CHUNK + (gidx // CHUNK)
    max_cnt = 0
    for c in range(N_CORES):
        s = slice(c * NP_CORE, (c + 1) * NP_CORE)
        max_cnt = max(max_cnt, np.bincount(all_bucket[s], minlength=NB).max())
    cap = (int(max_cnt) + SUBCAP - 1) // SUBCAP * SUBCAP

    if _graph_cache.get("cap") != cap:
        _graph_cache["nc"] = _build_graph(cap)
        _graph_cache["cap"] = cap
    nc = _graph_cache["nc"]
    cap128 = cap // P

    in_maps, orders, counts_l = [], [], []
    for c in range(N_CORES):
        s = slice(c * NP_CORE, (c + 1) * NP_CORE)
        u16, g16, order, counts = _shard_core(uidx[s], gidx[s], cap)
        orders.append(order)
        counts_l.append(counts)
        in_maps.append({
            "user_tab": user_rows,
            "goods_tab": goods_rows,
            "uidx": u16,
            "gidx": g16,
        })

    res = run_bass_kernel_spmd(nc, in_maps, core_ids=list(range(N_CORES)),
                               trace=TRACE)
    LAST_RESULT = res

    full = np.empty(N_PAIRS, np.float32)
    for c in range(N_CORES):
        out_arr = res.results[c]["out"]          # [P, NB*cap128]
        vals = np.empty(NP_CORE, np.float32)
        start = 0
        for b in range(NB):
            n = counts_l[c][b]
            # position i of bucket -> sub j = i//sc, then [i%128, j*(sc//128) + (i%sc)//128]
            sc = TAILCAP if b == NB - 1 else SUBCAP
            block = out_arr[:, b * cap128:(b + 1) * cap128]   # [128, cap128]
            # flatten to position order: [sub, within-sub col, partition]
            pos = block.reshape(P, cap // sc, sc // P).transpose(1, 2, 0).ravel()
            vals[start:start + n] = pos[:n]
            start += n
        dst = full[c * NP_CORE:(c + 1) * NP_CORE]
        dst[orders[c]] = vals
    return full.reshape(N_PAIRS, 1)



# revision 6
# speedup vs baseline: 1.1973x; 1.1973x over previous
"""Trainium2 Bass kernel for nn_ALS_net (embedding_lookup):
out[i] = dot(user_matrix[location[i,0], :], goods_matrix[:, location[i,1]])

Strategy (8 NeuronCores, SPMD):
  - Data-parallel over the 1M location pairs: 125K pairs per core.
  - Both factor tables replicated per core in HBM as [100000, 64] f32 row-major
    (goods transposed on host - layout marshalling).
  - dma_gather (custom SWDGE gather instruction) needs int16 row indices, so
    each table is split into 4 chunks of 25000 rows; pairs are bucketed by the
    (user-chunk, goods-chunk) combination -> 16 joint buckets per core. Within
    a bucket both gathers use dense int16 local indices; buckets are padded to
    a common capacity with trailing -1 (trimmed for free by the gather ucode).
    Each gather instruction is limited to 1024 indices (64 descriptors per
    SDMA lane fits one packet), so buckets issue ceil(cap/1024) sub-gathers.
  - Per bucket: gather user rows + goods rows (256B each) into SBUF,
    DVE elementwise multiply + free-dim reduce -> one f32 dot per pair.
  - Perf: SWDGE descriptor generation (~8ns/index) is the bottleneck, and
    each dma_gather's ucode runs on only the pair of gpsimd DSP cores
    selected by queue_num. Sub-gathers therefore round-robin over all 4
    SWDGE queues (num_swdge_queues=4) so descriptor generation for 4
    gathers runs concurrently on 4 distinct core pairs (~4x the serial
    rate: 2.34ms -> 0.59ms). 4096-index multi-packet sub-gathers amortize
    per-instruction dispatch; buckets triple-buffer in SBUF; idx tensors
    load via the Sync engine's HW-DGE queue so they overlap the gpsimd
    library load; the last bucket uses 1024-index sub-gathers to shrink
    the final DMA drain on the critical path. 0.58ms final.
  - Host inverts the bucket permutation when gluing shard outputs.
"""
import numpy as np

import concourse.bass as bass
import concourse.bacc as bacc
import concourse.mybir as mybir
from concourse.bass_utils import run_bass_kernel_spmd
from concourse.library_config import mlp

N_PAIRS = 1_000_000
K = 64
NROWS = 100_000          # index range per problem spec (both tables)
N_CHUNK = 4              # table chunks (int16 index reach)
CHUNK = NROWS // N_CHUNK  # 25000 rows per chunk
NB = N_CHUNK * N_CHUNK   # 16 joint buckets
N_CORES = 8
P = 128
NP_CORE = N_PAIRS // N_CORES          # 125000
SUBCAP = 4096            # indices per dma_gather (multi-packet, 4 queues)
TAILCAP = 1024           # last bucket: smaller gathers shrink final DMA drain

TRACE = False            # test.py sets True to capture a profile
LAST_RESULT = None       # BassKernelResults of the last run (for test.py)

_graph_cache = {}


def _build_graph(cap):
    """cap: per-bucket padded index capacity (multiple of SUBCAP)."""
    cap128 = cap // P
    cols16 = cap // 16
    nsub = cap // SUBCAP
    sub128 = SUBCAP // P      # 8 dst rows per sub-gather
    sub16 = SUBCAP // 16      # 64 idx cols per sub-gather
    nc = bacc.Bacc("TRN2", target_bir_lowering=False, debug=False,
                   enable_asserts=False, num_devices=N_CORES,
                   num_swdge_queues=4,
                   dynamic_dma_scratch_size=32768)
    user_d = nc.dram_tensor("user_tab", [NROWS, K], mybir.dt.float32, kind="ExternalInput").ap()
    goods_d = nc.dram_tensor("goods_tab", [NROWS, K], mybir.dt.float32, kind="ExternalInput").ap()
    uidx_d = nc.dram_tensor("uidx", [P, NB * cols16], mybir.dt.int16, kind="ExternalInput").ap()
    gidx_d = nc.dram_tensor("gidx", [P, NB * cols16], mybir.dt.int16, kind="ExternalInput").ap()
    out_d = nc.dram_tensor("out", [P, NB * cap128], mybir.dt.float32, kind="ExternalOutput").ap()

    with nc.Block() as block, \
         nc.sbuf_tensor("uidx_sb", [P, NB * cols16], mybir.dt.int16) as uidx_sb, \
         nc.sbuf_tensor("gidx_sb", [P, NB * cols16], mybir.dt.int16) as gidx_sb, \
         nc.sbuf_tensor("u_t", [P, 3, cap128, K], mybir.dt.float32) as u_t, \
         nc.sbuf_tensor("g_t", [P, 3, cap128, K], mybir.dt.float32) as g_t, \
         nc.sbuf_tensor("prod", [P, cap128, K], mybir.dt.float32) as prod, \
         nc.sbuf_tensor("out_sb", [P, NB * cap128], mybir.dt.float32) as out_sb, \
         nc.semaphore("io") as io, \
         nc.semaphore("gat0") as gat0, \
         nc.semaphore("gat1") as gat1, \
         nc.semaphore("gat2") as gat2, \
         nc.semaphore("cmp") as cmp, \
         nc.semaphore("red") as red:

        gat = (gat0, gat1, gat2)

        @block.sync
        def _(sync: bass.BassEngine):
            sync.dma_start(out=uidx_sb[:], in_=uidx_d[:]).then_inc(io, 16)
            sync.dma_start(out=gidx_sb[:], in_=gidx_d[:]).then_inc(io, 16)

        @block.gpsimd
        def _(gpsimd: bass.BassGpSimd):
            gpsimd.load_library(mlp)
            gpsimd.wait_ge(io, 16)   # uidx loaded; gidx DMA overlaps 1st desc-gen
            for b in range(NB):
                cu, cg = divmod(b, N_CHUNK)
                if b == 0:
                    pass
                elif b == 1:
                    gpsimd.wait_ge(io, 32)
                elif b >= 3:
                    gpsimd.wait_ge(cmp, b - 2)   # mul of bucket b-3 done
                for ti, (tab_ap, c, idx_sb, dst) in enumerate((
                        (user_d, cu, uidx_sb, u_t),
                        (goods_d, cg, gidx_sb, g_t))):
                    tab_c = tab_ap[c * CHUNK:(c + 1) * CHUNK, :]
                    if b == 0 and ti == 1:
                        gpsimd.wait_ge(io, 32)
                    sc = TAILCAP if b == NB - 1 else SUBCAP
                    s128, s16 = sc // P, sc // 16
                    for j in range(cap // sc):
                        qn = (ti * (cap // sc) + j) % 4
                        gpsimd.dma_gather(
                            dst[:, b % 3, j * s128:(j + 1) * s128],
                            tab_c,
                            idx_sb[:, b * cols16 + j * s16:
                                      b * cols16 + (j + 1) * s16],
                            sc, sc, K,
                            single_packet=False,
                            queue_num=qn,
                        ).then_inc(gat[b % 3], 16)
            gpsimd.wait_ge(red, NB)
            gpsimd.dma_start(out=out_d[:], in_=out_sb[:]).then_inc(io, 16)
            gpsimd.wait_ge(io, 48)

        @block.vector
        def _(vector: bass.BassVectorEngine):
            gat_tot = [0, 0, 0]
            for b in range(NB):
                sc = TAILCAP if b == NB - 1 else SUBCAP
                gat_tot[b % 3] += 2 * (cap // sc) * 16
                vector.wait_ge(gat[b % 3], gat_tot[b % 3])
                vector.tensor_tensor(out=prod[:], in0=u_t[:, b % 3], in1=g_t[:, b % 3],
                                     op=mybir.AluOpType.mult).then_inc(cmp, 1)
                vector.tensor_reduce(out=out_sb[:, b * cap128:(b + 1) * cap128],
                                     in_=prod[:], axis=mybir.AxisListType.X,
                                     op=mybir.AluOpType.add).then_inc(red, 1)
    nc.compile()
    return nc


def _shard_core(uidx, gidx, cap):
    """Bucket + sort one core's pairs. Returns (uidx16, gidx16, order, counts)."""
    cols16 = cap // 16
    a = uidx.astype(np.int64)
    g = gidx.astype(np.int64)
    bucket = (a // CHUNK) * N_CHUNK + (g // CHUNK)
    order = np.lexsort((a, bucket))          # by bucket, then user idx (HBM locality)
    counts = np.bincount(bucket, minlength=NB)
    assert counts.max() <= cap, (counts.max(), cap)
    au = (a[order] % CHUNK).astype(np.int16)
    ag = (g[order] % CHUNK).astype(np.int16)
    u16 = np.empty((P, NB * cols16), np.int16)
    g16 = np.empty((P, NB * cols16), np.int16)
    start = 0
    for b in range(NB):
        n = counts[b]
        for src, dst in ((au, u16), (ag, g16)):
            # pad with row 0 (NOT -1: an all-(-1) sub-gather crashes the ucode)
            flat = np.zeros(cap, np.int16)
            flat[:n] = src[start:start + n]
            # per sub-gather wrapped layout: position i of sub j lives at
            # [partition i%16 (replicated x8), col j*(sc//16) + i//16]
            sc = TAILCAP if b == NB - 1 else SUBCAP
            w = flat.reshape(cap // sc, sc // 16, 16)
            w = np.concatenate([w[j].T for j in range(cap // sc)], axis=1)
            dst[:, b * cols16:(b + 1) * cols16] = np.tile(w, (8, 1))
        start += n
    return u16, g16, order, counts


def kernel(location, user_matrix, goods_matrix):
    global LAST_RESULT
    loc = np.asarray(location)
    uidx = np.ascontiguousarray(loc[:, 0]).astype(np.int64, copy=False)
    gidx = np.ascontiguousarray(loc[:, 1]).astype(np.int64, copy=False)
    user_rows = np.ascontiguousarray(np.asarray(user_matrix)[:NROWS], dtype=np.float32)
    goods_rows = np.ascontiguousarray(np.asarray(goods_matrix).T, dtype=np.float32)

    # per-bucket capacity: max count over all cores/buckets, rounded to SUBCAP
    all_bucket = (uidx // CHUNK) * N_CHUNK + (gidx // CHUNK)
    max_cnt = 0
    for c in range(N_CORES):
        s = slice(c * NP_CORE, (c + 1) * NP_CORE)
        max_cnt = max(max_cnt, np.bincount(all_bucket[s], minlength=NB).max())
    cap = (int(max_cnt) + SUBCAP - 1) // SUBCAP * SUBCAP

    if _graph_cache.get("cap") != cap:
        _graph_cache["nc"] = _build_graph(cap)
        _graph_cache["cap"] = cap
    nc = _graph_cache["nc"]
    cap128 = cap // P

    in_maps, orders, counts_l = [], [], []
    for c in range(N_CORES):
        s = slice(c * NP_CORE, (c + 1) * NP_CORE)
        u16, g16, order, counts = _shard_core(uidx[s], gidx[s], cap)
        orders.append(order)
        counts_l.append(counts)
        in_maps.append({
            "user_tab": user_rows,
            "goods_tab": goods_rows,
            "uidx": u16,
            "gidx": g16,
        })

    res = run_bass_kernel_spmd(nc, in_maps, core_ids=list(range(N_CORES)),
                               trace=TRACE)
    LAST_RESULT = res

    full = np.empty(N_PAIRS, np.float32)
    for c in range(N_CORES):
        out_arr = res.results[c]["out"]          # [P, NB*cap128]
        vals = np.empty(NP_CORE, np.float32)
        start = 0
        for b in range(NB):
            n = counts_l[c][b]
            # position i of bucket -> sub j = i//sc, then [i%128, j*(sc//128) + (i%sc)//128]
            sc = TAILCAP if b == NB - 1 else SUBCAP
            block = out_arr[:, b * cap128:(b + 1) * cap128]   # [128, cap128]
            # flatten to position order: [sub, within-sub col, partition]
            pos = block.reshape(P, cap // sc, sc // P).transpose(1, 2, 0).ravel()
            vals[start:start + n] = pos[:n]
            start += n
        dst = full[c * NP_CORE:(c + 1) * NP_CORE]
        dst[orders[c]] = vals
    return full.reshape(N_PAIRS, 1)

